# revision 13
# baseline (speedup 1.0000x reference)
"""Differentiable random-forest layer (inference path) on 8 Trainium2 cores.

Computation (per reference):
    d     = sigmoid(einsum('bf,tfn->btn', x, W))        # [B, T, 255]
    route = prod_l where(IS_LEFT, d[..n..], 1-d[..n..]) # [B, T, 256]
    out   = clip(einsum('btl,tlc->bc', route, P) / T, 0, 1)

Shapes: B=4096, F=1024, T=10 trees, 255 nodes / 256 leaves, C=1000.

Sharding: data-parallel over batch. Each of the 8 cores handles 512 rows;
no collectives are needed (weights/probs are broadcast to every core).

Per-core pipeline (all matmuls bf16 inputs with fp32 PSUM accumulation):
  mm1   : d_logits[b,510] += xT[k,b].T @ W[k, tree-pair]   (PE)
  sig   : d = sigmoid(logits), dbar = sigmoid(-logits)     (ACT, psum->sbuf bf16)
  route : hierarchical doubling R_{l+1} = [R_l*d_l, R_l*dbar_l]  (DVE)
  transp: route [b,leaf] -> routeT [leaf,b] via batched XBAR DMA transpose
  mm2   : out[b,c] += routeT.T @ P[leaf-chunk]             (PE, over trees)
  store : out = relu(0.1 * psum)  -> DRAM                  (ACT + DMA)

The routing uses the "concat" (decision-bit-as-LSB) leaf ordering so every
DVE read/write is contiguous; the host pre-permutes W's node axis (per-layer
bit-reversal) and P's leaf axis (8-bit reversal) to compensate, which is free.
"""

from contextlib import ExitStack

import numpy as np
import ml_dtypes

import concourse.bass as bass
import concourse.bacc as bacc
import concourse.mybir as mybir
import concourse.tile as tile
from concourse.bass_utils import run_bass_kernel_spmd

N_CORES = 8
B, F, T, NODES, LEAFS, C = 4096, 1024, 10, 255, 256, 1000
B_LOC = B // N_CORES            # 512 batch rows per core
BCH = B_LOC // 128              # 4 batch chunks of 128
KF = F // 128                   # 8 contraction chunks for mm1
TP = T // 2                     # 5 tree-pairs (2 trees -> 510 psum cols)
N_LAYERS = 8

BF16 = mybir.dt.bfloat16
F32 = mybir.dt.float32
Sigmoid = mybir.ActivationFunctionType.Sigmoid
Relu = mybir.ActivationFunctionType.Relu


def _bitrev(x: int, bits: int) -> int:
    r = 0
    for _ in range(bits):
        r = (r << 1) | (x & 1)
        x >>= 1
    return r


# Node-axis permutation: d'[.., off+q] = d[.., off+bitrev_l(q)] per layer l
NODE_PERM = np.empty(NODES, dtype=np.int64)
for _l in range(N_LAYERS):
    _off = (1 << _l) - 1
    for _q in range(1 << _l):
        NODE_PERM[_off + _q] = _off + _bitrev(_q, _l)
# Leaf-axis permutation: P'[t, q, :] = P[t, bitrev_8(q), :]
LEAF_PERM = np.array([_bitrev(q, N_LAYERS) for q in range(LEAFS)], dtype=np.int64)


def build_program() -> bass.Bass:
    nc = bacc.Bacc()

    xT = nc.dram_tensor("xT", [KF, 128, B_LOC], BF16, kind="ExternalInput")
    w = nc.dram_tensor("w", [KF, 128, T * NODES], BF16, kind="ExternalInput")
    p = nc.dram_tensor("p", [2, 128, T * C], BF16, kind="ExternalInput")
    out = nc.dram_tensor("out", [B_LOC, C], F32, kind="ExternalOutput")

    with tile.TileContext(nc) as tc, ExitStack() as ctx:
        resident = ctx.enter_context(tc.tile_pool(name="resident", bufs=1))
        x_all = resident.tile([128, KF, B_LOC], BF16, tag="x_all", name="x_all")
        w_all = resident.tile([128, KF, T * NODES], BF16, tag="w_all", name="w_all")
        p_all = resident.tile([128, 2, T * C], BF16, tag="p_all", name="p_all")
        # k=0 split out so mm1 can start as soon as its first chunks land;
        # few big DMAs keep the sync queue's trigger overhead low.
        nc.sync.dma_start(w_all[:, 0:1, :], w[0:1].rearrange("k p n -> p k n"))
        nc.sync.dma_start(x_all[:, 0:1, :], xT[0:1].rearrange("k p n -> p k n"))
        nc.sync.dma_start(w_all[:, 1:KF, :], w[1:KF].rearrange("k p n -> p k n"))
        nc.sync.dma_start(x_all[:, 1:KF, :], xT[1:KF].rearrange("k p n -> p k n"))
        nc.sync.dma_start(p_all[:, :, :], p.rearrange("k p n -> p k n"))
        xT_sb = [x_all[:, k, :] for k in range(KF)]
        w_sb = [w_all[:, k, :] for k in range(KF)]
        p_sb = [p_all[:, kc, :] for kc in range(2)]

        dpool = ctx.enter_context(tc.tile_pool(name="dps", bufs=1, space="PSUM"))
        opool = ctx.enter_context(tc.tile_pool(name="ops", bufs=2, space="PSUM"))
        work = ctx.enter_context(tc.tile_pool(name="work", bufs=2))

        for bi in range(BCH):
            bsl = bass.ts(bi, 128)

            # ---- mm1: d logits for all 10 trees, this batch chunk ----
            dps = [
                dpool.tile([128, 2, NODES], F32, tag=f"dps{j}", name=f"dps{j}")
                for j in range(TP)
            ]
            for k in range(KF):
                lhs = xT_sb[k][:, bsl]
                for j in range(TP):
                    nc.tensor.matmul(
                        dps[j][:, :, :],
                        lhs,
                        w_sb[k][:, j * 2 * NODES : (j + 1) * 2 * NODES],
                        start=(k == 0),
                        stop=(k == KF - 1),
                    )

            # ---- sigmoid: ddb[0]=d, ddb[1]=sigmoid(-x)=1-d, psum -> sbuf bf16 ----
            ddb = work.tile([128, 2, T, NODES], BF16, tag="ddb", name="ddb")
            for j in range(TP):
                nc.scalar.activation(
                    ddb[:, 0, 2 * j : 2 * j + 2, :], dps[j][:, :, :], Sigmoid
                )
                nc.scalar.activation(
                    ddb[:, 1, 2 * j : 2 * j + 2, :], dps[j][:, :, :], Sigmoid, scale=-1.0
                )

            # ---- routing: hierarchical doubling, concat ordering ----
            # R_{l+1}[0:w]  = R_l[0:w] * d_l   (decision bit 0 -> left)
            # R_{l+1}[w:2w] = R_l[0:w] * dbar_l
            # One DVE op per layer: out [t, half, w], in0 broadcast over half
            # (step-0 dim), in1 = ddb [t, half, w].
            Ra = work.tile([128, T, LEAFS], BF16, tag="Ra", name="Ra")
            Rb = work.tile([128, T, LEAFS], BF16, tag="Rb", name="Rb")
            routeC = work.tile([128, 2, T, 128], BF16, tag="routeC", name="routeC")

            nc.vector.tensor_copy(Ra[:, :, 0:1], ddb[:, 0, :, 0:1])
            nc.vector.tensor_copy(Ra[:, :, 1:2], ddb[:, 1, :, 0:1])
            cur, nxt = Ra, Rb
            for l in range(1, N_LAYERS):
                w_l = 1 << l          # prefixes at layer l
                off = w_l - 1         # first node index of layer l
                if l < N_LAYERS - 1:
                    lo, hi = nxt[:, :, 0:w_l], nxt[:, :, w_l : 2 * w_l]
                else:
                    # last layer: write straight into the transpose-ready
                    # [leaf-chunk, tree, leaf-low] layout
                    lo, hi = routeC[:, 0, :, :], routeC[:, 1, :, :]
                nc.vector.tensor_mul(lo, cur[:, :, 0:w_l], ddb[:, 0, :, off : off + w_l])
                nc.vector.tensor_mul(hi, cur[:, :, 0:w_l], ddb[:, 1, :, off : off + w_l])
                cur, nxt = nxt, cur

            # ---- transpose: route [b, leaf] -> routeT [leaf, b], per kc ----
            rT = [
                work.tile([128, T, 128], BF16, tag=f"rT{kc}", name=f"rT{kc}", bufs=4)
                for kc in range(2)
            ]
            nc.sync.dma_start_transpose(rT[0][:, :, :], routeC[:, 0])
            nc.sync.dma_start_transpose(rT[1][:, :, :], routeC[:, 1])

            # ---- mm2: out[b, c] += routeT.T @ P, accumulated over trees ----
            osb = work.tile([128, C], F32, tag="osb", name="osb")
            for n0, nsz in ((0, 512), (512, C - 512)):
                ops = opool.tile([128, 512], F32, tag="ops", name="ops")
                for t_ in range(T):
                    for kc in range(2):
                        nc.tensor.matmul(
                            ops[:, 0:nsz],
                            rT[kc][:, t_, :],
                            p_sb[kc][:, t_ * C + n0 : t_ * C + n0 + nsz],
                            start=(t_ == 0 and kc == 0),
                            stop=(t_ == T - 1 and kc == 1),
                        )
                # mean over trees (x0.1) + clip lower bound; upper bound is
                # provably inactive (outputs <= max(P) ~ 2e-3)
                nc.scalar.activation(osb[:, n0 : n0 + nsz], ops[:, 0:nsz], Relu, scale=1.0 / T)
                nc.scalar.dma_start(out[bsl, n0 : n0 + nsz], osb[:, n0 : n0 + nsz])

    nc.finalize()
    return nc


_CACHED_NC = None


def _get_nc() -> bass.Bass:
    global _CACHED_NC
    if _CACHED_NC is None:
        _CACHED_NC = build_program()
    return _CACHED_NC


def _prep_inputs(l_input, cnn_w, final_probabilities):
    bf = ml_dtypes.bfloat16
    x = np.ascontiguousarray(np.asarray(l_input, dtype=np.float32))
    W = np.asarray(cnn_w, dtype=np.float32)[:, :, NODE_PERM]
    P = np.asarray(final_probabilities, dtype=np.float32)[:, LEAF_PERM, :]

    # x [B, F] -> xT [KF, 128, B] (transposed, contraction-chunk major)
    xT = np.ascontiguousarray(x.T).astype(bf).reshape(KF, 128, B)
    # W [T, F, N] -> [F, T, N] -> [KF, 128, T*N]
    Wr = np.ascontiguousarray(W.transpose(1, 0, 2)).astype(bf).reshape(KF, 128, T * NODES)
    # P [T, 256, C] -> [leaf-chunk, 128, T*C]
    Pr = np.ascontiguousarray(
        P.reshape(T, 2, 128, C).transpose(1, 2, 0, 3)
    ).astype(bf).reshape(2, 128, T * C)
    return xT, Wr, Pr


def _run(inputs, trace=False, trace_cores=None):
    xT, Wr, Pr = _prep_inputs(
        inputs["l_input"], inputs["cnn_w"], inputs["final_probabilities"]
    )
    in_maps = [
        {
            "xT": np.ascontiguousarray(xT[:, :, c * B_LOC : (c + 1) * B_LOC]),
            "w": Wr,
            "p": Pr,
        }
        for c in range(N_CORES)
    ]
    res = run_bass_kernel_spmd(
        _get_nc(),
        in_maps,
        core_ids=list(range(N_CORES)),
        trace=trace,
        trace_cores=trace_cores,
    )
    out = np.concatenate([res.results[c]["out"] for c in range(N_CORES)], axis=0)
    return out, res


def kernel(**inputs) -> np.ndarray:
    out, _ = _run(inputs)
    return out


# revision 30
# speedup vs baseline: 1.3783x; 1.3783x over previous
"""Differentiable random-forest layer (inference path) on 8 Trainium2 cores.

Computation (per reference):
    d     = sigmoid(einsum('bf,tfn->btn', x, W))        # [B, T, 255]
    route = prod_l where(IS_LEFT, d[..n..], 1-d[..n..]) # [B, T, 256]
    out   = clip(einsum('btl,tlc->bc', route, P) / T, 0, 1)

Shapes: B=4096, F=1024, T=10 trees, 255 nodes / 256 leaves, C=1000.

Sharding: data-parallel over batch. Each of the 8 cores handles 512 rows;
no collectives are needed (weights/probs are broadcast to every core).

Per-core pipeline (all matmuls bf16 inputs with fp32 PSUM accumulation):
  mm1   : d_logits[b,510] += xT[k,b].T @ W[k, tree-pair]   (PE)
  sig   : d = sigmoid(logits), dbar = sigmoid(-logits)     (ACT, psum->sbuf bf16)
  route : hierarchical doubling R_{l+1} = [R_l*d_l, R_l*dbar_l]  (DVE)
  transp: route [b,leaf] -> routeT [leaf,b] via batched XBAR DMA transpose
  mm2   : out[b,c] += routeT.T @ P[leaf-chunk]             (PE, over trees)
  store : out = max(0.1 * psum, 0) -> DRAM                 (DVE + DMA)

The routing uses the "concat" (decision-bit-as-LSB) leaf ordering so every
DVE read/write is contiguous; the host pre-permutes W's node axis (per-layer
bit-reversal) and P's leaf axis (8-bit reversal) to compensate, which is free.
"""

from contextlib import ExitStack

import numpy as np
import ml_dtypes

import concourse.bass as bass
import concourse.bacc as bacc
import concourse.mybir as mybir
import concourse.tile as tile
from concourse.bass_utils import run_bass_kernel_spmd

N_CORES = 8
B, F, T, NODES, LEAFS, C = 4096, 1024, 10, 255, 256, 1000
B_LOC = B // N_CORES            # 512 batch rows per core
BCH = B_LOC // 128              # 4 batch chunks of 128
KF = F // 128                   # 8 contraction chunks for mm1
TP = T // 2                     # 5 tree-pairs (2 trees -> 510 psum cols)
N_LAYERS = 8

BF16 = mybir.dt.bfloat16
F32 = mybir.dt.float32
Sigmoid = mybir.ActivationFunctionType.Sigmoid


def _bitrev(x: int, bits: int) -> int:
    r = 0
    for _ in range(bits):
        r = (r << 1) | (x & 1)
        x >>= 1
    return r


# Node-axis permutation: d'[.., off+q] = d[.., off+bitrev_l(q)] per layer l
NODE_PERM = np.empty(NODES, dtype=np.int64)
for _l in range(N_LAYERS):
    _off = (1 << _l) - 1
    for _q in range(1 << _l):
        NODE_PERM[_off + _q] = _off + _bitrev(_q, _l)
# Leaf-axis permutation: P'[t, q, :] = P[t, bitrev_8(q), :]
LEAF_PERM = np.array([_bitrev(q, N_LAYERS) for q in range(LEAFS)], dtype=np.int64)


def build_program() -> bass.Bass:
    nc = bacc.Bacc()

    xT = nc.dram_tensor("xT", [KF, 128, B_LOC], BF16, kind="ExternalInput")
    # W is j-major: one contiguous block per tree-pair j covering all KF chunks
    w = nc.dram_tensor("w", [TP, 128, KF * 2 * NODES], BF16, kind="ExternalInput")
    p = nc.dram_tensor("p", [2, 128, T * C], BF16, kind="ExternalInput")
    out = nc.dram_tensor("out", [B_LOC, C], F32, kind="ExternalOutput")

    with tile.TileContext(nc) as tc, ExitStack() as ctx:
        resident = ctx.enter_context(tc.tile_pool(name="resident", bufs=1))
        x_all = resident.tile([128, KF, B_LOC], BF16, tag="x_all", name="x_all")
        w_all = resident.tile([128, TP, KF, 2 * NODES], BF16, tag="w_all", name="w_all")
        p_all = resident.tile([128, 2, T * C], BF16, tag="p_all", name="p_all")
        # Load order tuned so mm1(b0, j) can start as each j-block lands:
        # x k=0, W j=0, rest of x, W j=1.., then P (needed only by mm2).
        nc.sync.dma_start(x_all[:, 0:1, :], xT[0:1].rearrange("k p n -> p k n"))
        nc.sync.dma_start(w_all[:, 0, :, :], w[0])
        nc.sync.dma_start(x_all[:, 1:KF, :], xT[1:KF].rearrange("k p n -> p k n"))
        for j in range(1, TP):
            nc.sync.dma_start(w_all[:, j, :, :], w[j])
        nc.sync.dma_start(p_all[:, :, :], p.rearrange("k p n -> p k n"))
        xT_sb = [x_all[:, k, :] for k in range(KF)]
        p_sb = [p_all[:, kc, :] for kc in range(2)]

        dpool = ctx.enter_context(tc.tile_pool(name="dps", bufs=1, space="PSUM"))
        opool = ctx.enter_context(tc.tile_pool(name="ops", bufs=3, space="PSUM"))
        work = ctx.enter_context(tc.tile_pool(name="work", bufs=2))

        # ---- PE warmup: the first ~17us are DMA-bound, so the PE would sit
        # idle and its HAM clock gate stays at half speed for the first real
        # matmuls. Run ~3.5us of dummy matmuls on a zeroed tile so the PE is
        # at full clock when the weights land. ----
        warm_in = work.tile([128, 128], BF16, tag="warm", name="warm_in", bufs=1)
        nc.gpsimd.memset(warm_in[:, :], 0.0)
        warm_ps = opool.tile([128, 128], F32, tag="warm", name="warm_ps", bufs=1)

        def warm_mms(n):
            for _ in range(n):
                nc.tensor.matmul(warm_ps[:, :], warm_in[:, :], warm_in[:, :])

        warm_mms(88)

        def emit_mm2(rT, bsl, nchunks=((0, 512), (512, C - 512))):
            # mm2: out[b, c] += routeT.T @ P, accumulated over trees, then
            # mean over trees (x0.1) + clip lower bound (upper bound provably
            # inactive: outputs <= max(P) ~ 2e-3). Scale/relu on DVE so the
            # ACT engine stays dedicated to the sigmoid pipeline.
            osb = work.tile([128, C], F32, tag="osb", name="osb")
            for n0, nsz in nchunks:
                ops = opool.tile([128, 512], F32, tag="ops", name="ops")
                for t_ in range(T):
                    for kc in range(2):
                        nc.tensor.matmul(
                            ops[:, 0:nsz],
                            rT[kc][:, t_, :],
                            p_sb[kc][:, t_ * C + n0 : t_ * C + n0 + nsz],
                            start=(t_ == 0 and kc == 0),
                            stop=(t_ == T - 1 and kc == 1),
                        )
                nc.vector.tensor_scalar(
                    osb[:, n0 : n0 + nsz], ops[:, 0:nsz], 1.0 / T, 0.0,
                    mybir.AluOpType.mult, mybir.AluOpType.max,
                )
                nc.sync.dma_start(out[bsl, n0 : n0 + nsz], osb[:, n0 : n0 + nsz])

        def emit_mm1_j(bi, j, ddb):
            # d logits for tree-pair j of chunk bi, then sigmoids into ddb
            dps = dpool.tile([128, 2, NODES], F32, tag="dps", name="dps", bufs=3)
            for k in range(KF):
                nc.tensor.matmul(
                    dps[:, :, :],
                    xT_sb[k][:, bass.ts(bi, 128)],
                    w_all[:, j, k, :],
                    start=(k == 0),
                    stop=(k == KF - 1),
                )
            # sigmoid: ddb[0]=d, ddb[1]=sigmoid(-x)=1-d, psum -> sbuf bf16
            nc.scalar.activation(ddb[:, 0, 2 * j : 2 * j + 2, :], dps[:, :, :], Sigmoid)
            nc.scalar.activation(
                ddb[:, 1, 2 * j : 2 * j + 2, :], dps[:, :, :], Sigmoid, scale=-1.0
            )

        def emit_routing(ddb):
            # ---- routing: hierarchical doubling, concat ordering ----
            # R_{l+1}[0:w]  = R_l[0:w] * d_l   (decision bit 0 -> left)
            # R_{l+1}[w:2w] = R_l[0:w] * dbar_l
            Ra = work.tile([128, T, LEAFS], BF16, tag="Ra", name="Ra")
            Rb = work.tile([128, T, LEAFS], BF16, tag="Rb", name="Rb")
            routeC = work.tile([128, 2, T, 128], BF16, tag="routeC", name="routeC")
            nc.vector.tensor_copy(Ra[:, :, 0:1], ddb[:, 0, :, 0:1])
            nc.vector.tensor_copy(Ra[:, :, 1:2], ddb[:, 1, :, 0:1])
            cur, nxt = Ra, Rb
            for l in range(1, N_LAYERS):
                w_l = 1 << l          # prefixes at layer l
                off = w_l - 1         # first node index of layer l
                if l < N_LAYERS - 1:
                    lo, hi = nxt[:, :, 0:w_l], nxt[:, :, w_l : 2 * w_l]
                else:
                    # last layer: write straight into the transpose-ready
                    # [leaf-chunk, tree, leaf-low] layout
                    lo, hi = routeC[:, 0, :, :], routeC[:, 1, :, :]
                nc.vector.tensor_mul(lo, cur[:, :, 0:w_l], ddb[:, 0, :, off : off + w_l])
                nc.vector.tensor_mul(hi, cur[:, :, 0:w_l], ddb[:, 1, :, off : off + w_l])
                cur, nxt = nxt, cur
            # transpose: route [b, leaf] -> routeT [leaf, b], per leaf-chunk
            rT = [
                work.tile([128, T, 128], BF16, tag=f"rT{kc}", name=f"rT{kc}", bufs=4)
                for kc in range(2)
            ]
            nc.sync.dma_start_transpose(rT[0][:, :, :], routeC[:, 0])
            nc.sync.dma_start_transpose(rT[1][:, :, :], routeC[:, 1])
            return rT

        # Emission order = desired per-engine instruction order. Chunks b0/b1
        # are interleaved at the tree-pair level so the PE has enough ready
        # work while the W blocks are still streaming in from HBM; afterwards
        # mm1 and mm2 of consecutive chunks alternate so each chunk's
        # sigmoid/routing/transpose chain hides under the other's PE work.
        ddb0 = work.tile([128, 2, T, NODES], BF16, tag="ddb", name="ddb0", bufs=3)
        ddb1 = work.tile([128, 2, T, NODES], BF16, tag="ddb", name="ddb1", bufs=3)
        for j in range(TP):
            emit_mm1_j(0, j, ddb0)
            emit_mm1_j(1, j, ddb1)
        rT0 = emit_routing(ddb0)
        ddb2 = work.tile([128, 2, T, NODES], BF16, tag="ddb", name="ddb2", bufs=3)
        for j in range(TP):
            emit_mm1_j(2, j, ddb2)
        rT1 = emit_routing(ddb1)
        emit_mm2(rT0, bass.ts(0, 128))
        ddb3 = work.tile([128, 2, T, NODES], BF16, tag="ddb", name="ddb3", bufs=3)
        for j in range(TP):
            emit_mm1_j(3, j, ddb3)
        rT2 = emit_routing(ddb2)
        emit_mm2(rT1, bass.ts(1, 128))
        rT3 = emit_routing(ddb3)
        emit_mm2(rT2, bass.ts(2, 128))
        # final chunk: finer output blocks so the last relu+store tail is short
        emit_mm2(rT3, bass.ts(3, 128), nchunks=((0, 256), (256, 256), (512, 256), (768, C - 768)))

    nc.finalize()
    return nc


_CACHED_NC = None


def _get_nc() -> bass.Bass:
    global _CACHED_NC
    if _CACHED_NC is None:
        _CACHED_NC = build_program()
    return _CACHED_NC


def _prep_inputs(l_input, cnn_w, final_probabilities):
    bf = ml_dtypes.bfloat16
    x = np.ascontiguousarray(np.asarray(l_input, dtype=np.float32))
    W = np.asarray(cnn_w, dtype=np.float32)[:, :, NODE_PERM]
    P = np.asarray(final_probabilities, dtype=np.float32)[:, LEAF_PERM, :]

    # x [B, F] -> xT [KF, 128, B] (transposed, contraction-chunk major)
    xT = np.ascontiguousarray(x.T).astype(bf).reshape(KF, 128, B)
    # W [T, F, N] -> [F, T, N] -> [KF, 128, TP, 510] -> j-major [TP, 128, KF*510]
    Wr = (
        np.ascontiguousarray(W.transpose(1, 0, 2))
        .astype(bf)
        .reshape(KF, 128, TP, 2 * NODES)
        .transpose(2, 1, 0, 3)
        .reshape(TP, 128, KF * 2 * NODES)
    )
    Wr = np.ascontiguousarray(Wr)
    # P [T, 256, C] -> [leaf-chunk, 128, T*C]
    Pr = np.ascontiguousarray(
        P.reshape(T, 2, 128, C).transpose(1, 2, 0, 3)
    ).astype(bf).reshape(2, 128, T * C)
    return xT, Wr, Pr


def _run(inputs, trace=False, trace_cores=None):
    xT, Wr, Pr = _prep_inputs(
        inputs["l_input"], inputs["cnn_w"], inputs["final_probabilities"]
    )
    in_maps = [
        {
            "xT": np.ascontiguousarray(xT[:, :, c * B_LOC : (c + 1) * B_LOC]),
            "w": Wr,
            "p": Pr,
        }
        for c in range(N_CORES)
    ]
    last_err = None
    for attempt in range(3):
        try:
            res = run_bass_kernel_spmd(
                _get_nc(),
                in_maps,
                core_ids=list(range(N_CORES)),
                trace=trace,
                trace_cores=trace_cores,
            )
            break
        except Exception as e:  # transient NRT device errors: retry
            last_err = e
            if attempt == 2:
                raise
            import time as _time

            _time.sleep(5)
    out = np.concatenate([res.results[c]["out"] for c in range(N_CORES)], axis=0)
    return out, res


def kernel(**inputs) -> np.ndarray:
    out, _ = _run(inputs)
    return out


# revision 35
# speedup vs baseline: 1.4098x; 1.0228x over previous
"""Differentiable random-forest layer (inference path) on 8 Trainium2 cores.

Computation (per reference):
    d     = sigmoid(einsum('bf,tfn->btn', x, W))        # [B, T, 255]
    route = prod_l where(IS_LEFT, d[..n..], 1-d[..n..]) # [B, T, 256]
    out   = clip(einsum('btl,tlc->bc', route, P) / T, 0, 1)

Shapes: B=4096, F=1024, T=10 trees, 255 nodes / 256 leaves, C=1000.

Sharding: data-parallel over batch. Each of the 8 cores handles 512 rows;
no collectives are needed (weights/probs are broadcast to every core).

Per-core pipeline (all matmuls bf16 inputs with fp32 PSUM accumulation):
  mm1   : d_logits[b,510] += xT[k,b].T @ W[k, tree-pair]   (PE)
  sig   : d = sigmoid(logits), dbar = sigmoid(-logits)     (ACT, psum->sbuf bf16)
  route : hierarchical doubling R_{l+1} = [R_l*d_l, R_l*dbar_l]  (DVE)
  transp: route [b,leaf] -> routeT [leaf,b] via batched XBAR DMA transpose
  mm2   : out[b,c] += routeT.T @ P[leaf-chunk]             (PE, over trees)
  store : out = max(0.1 * psum, 0) -> DRAM                 (DVE + DMA)

The routing uses the "concat" (decision-bit-as-LSB) leaf ordering so every
DVE read/write is contiguous; the host pre-permutes W's node axis (per-layer
bit-reversal) and P's leaf axis (8-bit reversal) to compensate, which is free.
"""

from contextlib import ExitStack

import numpy as np
import ml_dtypes

import concourse.bass as bass
import concourse.bacc as bacc
import concourse.mybir as mybir
import concourse.tile as tile
from concourse.bass_utils import run_bass_kernel_spmd

N_CORES = 8
B, F, T, NODES, LEAFS, C = 4096, 1024, 10, 255, 256, 1000
B_LOC = B // N_CORES            # 512 batch rows per core
BCH = B_LOC // 128              # 4 batch chunks of 128
KF = F // 128                   # 8 contraction chunks for mm1
TP = T // 2                     # 5 tree-pairs (2 trees -> 510 psum cols)
N_LAYERS = 8

BF16 = mybir.dt.bfloat16
F32 = mybir.dt.float32
Sigmoid = mybir.ActivationFunctionType.Sigmoid


def _bitrev(x: int, bits: int) -> int:
    r = 0
    for _ in range(bits):
        r = (r << 1) | (x & 1)
        x >>= 1
    return r


# Node-axis permutation: d'[.., off+q] = d[.., off+bitrev_l(q)] per layer l
NODE_PERM = np.empty(NODES, dtype=np.int64)
for _l in range(N_LAYERS):
    _off = (1 << _l) - 1
    for _q in range(1 << _l):
        NODE_PERM[_off + _q] = _off + _bitrev(_q, _l)
# Leaf-axis permutation: P'[t, q, :] = P[t, bitrev_8(q), :]
LEAF_PERM = np.array([_bitrev(q, N_LAYERS) for q in range(LEAFS)], dtype=np.int64)


def build_program() -> bass.Bass:
    nc = bacc.Bacc()

    xT = nc.dram_tensor("xT", [KF, 128, B_LOC], BF16, kind="ExternalInput")
    # W is j-major: one contiguous block per tree-pair j covering all KF chunks
    w = nc.dram_tensor("w", [TP, 128, KF * 2 * NODES], BF16, kind="ExternalInput")
    p = nc.dram_tensor("p", [2, 128, T * C], BF16, kind="ExternalInput")
    out = nc.dram_tensor("out", [B_LOC, C], F32, kind="ExternalOutput")

    with tile.TileContext(nc) as tc, ExitStack() as ctx:
        resident = ctx.enter_context(tc.tile_pool(name="resident", bufs=1))
        x_all = resident.tile([128, KF, B_LOC], BF16, tag="x_all", name="x_all")
        w_all = resident.tile([128, TP, KF, 2 * NODES], BF16, tag="w_all", name="w_all")
        p_all = resident.tile([128, 2, T * C], BF16, tag="p_all", name="p_all")
        # Load order tuned so mm1(b0, j) can start as each j-block lands:
        # x k=0, W j=0, rest of x, W j=1.., then P (needed only by mm2).
        nc.sync.dma_start(x_all[:, 0:1, :], xT[0:1].rearrange("k p n -> p k n"))
        nc.sync.dma_start(w_all[:, 0, :, :], w[0])
        nc.sync.dma_start(x_all[:, 1:KF, :], xT[1:KF].rearrange("k p n -> p k n"))
        for j in range(1, TP):
            nc.sync.dma_start(w_all[:, j, :, :], w[j])
        nc.sync.dma_start(p_all[:, :, :], p.rearrange("k p n -> p k n"))
        xT_sb = [x_all[:, k, :] for k in range(KF)]
        p_sb = [p_all[:, kc, :] for kc in range(2)]

        dpool = ctx.enter_context(tc.tile_pool(name="dps", bufs=1, space="PSUM"))
        opool = ctx.enter_context(tc.tile_pool(name="ops", bufs=3, space="PSUM"))
        work = ctx.enter_context(tc.tile_pool(name="work", bufs=2))

        # ---- PE warmup: the first ~17us are DMA-bound, so the PE would sit
        # idle and its HAM clock gate stays at half speed for the first real
        # matmuls. Run ~3.5us of dummy matmuls on a zeroed tile so the PE is
        # at full clock when the weights land. ----
        warm_in = work.tile([128, 128], BF16, tag="warm", name="warm_in", bufs=1)
        nc.vector.memset(warm_in[:, :], 0.0)
        warm_ps = opool.tile([128, 128], F32, tag="warm", name="warm_ps", bufs=1)

        def warm_mms(n):
            for _ in range(n):
                nc.tensor.matmul(warm_ps[:, :], warm_in[:, :], warm_in[:, :])

        warm_mms(88)

        def emit_mm2(rT, bsl, nchunks=((0, 512), (512, C - 512))):
            # mm2: out[b, c] += routeT.T @ (P/T), accumulated over trees.
            # The 1/T mean is folded into P on the host; the reference clip
            # is provably inactive (all terms nonneg, outputs <= max(P) ~2e-4
            # of 1.0), so the fp32 PSUM result IS the output: DMA it straight
            # to DRAM with no elementwise epilogue.
            osb = work.tile([128, C], F32, tag="osb", name="osb")
            for n0, nsz in nchunks:
                ops = opool.tile([128, 512], F32, tag="ops", name="ops")
                for t_ in range(T):
                    for kc in range(2):
                        nc.tensor.matmul(
                            ops[:, 0:nsz],
                            rT[kc][:, t_, :],
                            p_sb[kc][:, t_ * C + n0 : t_ * C + n0 + nsz],
                            start=(t_ == 0 and kc == 0),
                            stop=(t_ == T - 1 and kc == 1),
                        )
                nc.vector.tensor_copy(osb[:, n0 : n0 + nsz], ops[:, 0:nsz])
                nc.sync.dma_start(out[bsl, n0 : n0 + nsz], osb[:, n0 : n0 + nsz])

        def emit_mm1_j(bi, j, ddb):
            # d logits for tree-pair j of chunk bi, then sigmoids into ddb
            dps = dpool.tile([128, 2, NODES], F32, tag="dps", name="dps", bufs=3)
            for k in range(KF):
                nc.tensor.matmul(
                    dps[:, :, :],
                    xT_sb[k][:, bass.ts(bi, 128)],
                    w_all[:, j, k, :],
                    start=(k == 0),
                    stop=(k == KF - 1),
                )
            # sigmoid: ddb[0]=d, ddb[1]=sigmoid(-x)=1-d, psum -> sbuf bf16
            nc.scalar.activation(ddb[:, 0, 2 * j : 2 * j + 2, :], dps[:, :, :], Sigmoid)
            nc.scalar.activation(
                ddb[:, 1, 2 * j : 2 * j + 2, :], dps[:, :, :], Sigmoid, scale=-1.0
            )

        def emit_routing(ddb):
            # ---- routing: hierarchical doubling, concat ordering ----
            # R_{l+1}[0:w]  = R_l[0:w] * d_l   (decision bit 0 -> left)
            # R_{l+1}[w:2w] = R_l[0:w] * dbar_l
            Ra = work.tile([128, T, LEAFS], BF16, tag="Ra", name="Ra")
            Rb = work.tile([128, T, LEAFS], BF16, tag="Rb", name="Rb")
            routeC = work.tile([128, 2, T, 128], BF16, tag="routeC", name="routeC")
            nc.vector.tensor_copy(Ra[:, :, 0:1], ddb[:, 0, :, 0:1])
            nc.vector.tensor_copy(Ra[:, :, 1:2], ddb[:, 1, :, 0:1])
            cur, nxt = Ra, Rb
            for l in range(1, N_LAYERS):
                w_l = 1 << l          # prefixes at layer l
                off = w_l - 1         # first node index of layer l
                if l < N_LAYERS - 1:
                    lo, hi = nxt[:, :, 0:w_l], nxt[:, :, w_l : 2 * w_l]
                else:
                    # last layer: write straight into the transpose-ready
                    # [leaf-chunk, tree, leaf-low] layout
                    lo, hi = routeC[:, 0, :, :], routeC[:, 1, :, :]
                nc.vector.tensor_mul(lo, cur[:, :, 0:w_l], ddb[:, 0, :, off : off + w_l])
                nc.vector.tensor_mul(hi, cur[:, :, 0:w_l], ddb[:, 1, :, off : off + w_l])
                cur, nxt = nxt, cur
            # transpose: route [b, leaf] -> routeT [leaf, b], per leaf-chunk
            rT = [
                work.tile([128, T, 128], BF16, tag=f"rT{kc}", name=f"rT{kc}", bufs=4)
                for kc in range(2)
            ]
            nc.sync.dma_start_transpose(rT[0][:, :, :], routeC[:, 0])
            nc.sync.dma_start_transpose(rT[1][:, :, :], routeC[:, 1])
            return rT

        # Emission order = desired per-engine instruction order. Chunks b0/b1
        # are interleaved at the tree-pair level so the PE has enough ready
        # work while the W blocks are still streaming in from HBM; afterwards
        # mm1 and mm2 of consecutive chunks alternate so each chunk's
        # sigmoid/routing/transpose chain hides under the other's PE work.
        ddb0 = work.tile([128, 2, T, NODES], BF16, tag="ddb", name="ddb0", bufs=3)
        ddb1 = work.tile([128, 2, T, NODES], BF16, tag="ddb", name="ddb1", bufs=3)
        for j in range(TP):
            emit_mm1_j(0, j, ddb0)
            emit_mm1_j(1, j, ddb1)
        rT0 = emit_routing(ddb0)
        ddb2 = work.tile([128, 2, T, NODES], BF16, tag="ddb", name="ddb2", bufs=3)
        for j in range(TP):
            emit_mm1_j(2, j, ddb2)
        rT1 = emit_routing(ddb1)
        emit_mm2(rT0, bass.ts(0, 128))
        ddb3 = work.tile([128, 2, T, NODES], BF16, tag="ddb", name="ddb3", bufs=3)
        for j in range(TP):
            emit_mm1_j(3, j, ddb3)
        rT2 = emit_routing(ddb2)
        emit_mm2(rT1, bass.ts(1, 128))
        rT3 = emit_routing(ddb3)
        emit_mm2(rT2, bass.ts(2, 128))
        # final chunk: finer output blocks so the last relu+store tail is short
        emit_mm2(rT3, bass.ts(3, 128), nchunks=((0, 256), (256, 256), (512, 256), (768, C - 768)))

    nc.finalize()
    return nc


_CACHED_NC = None


def _get_nc() -> bass.Bass:
    global _CACHED_NC
    if _CACHED_NC is None:
        _CACHED_NC = build_program()
    return _CACHED_NC


def _prep_inputs(l_input, cnn_w, final_probabilities):
    bf = ml_dtypes.bfloat16
    x = np.ascontiguousarray(np.asarray(l_input, dtype=np.float32))
    W = np.asarray(cnn_w, dtype=np.float32)[:, :, NODE_PERM]
    # fold the 1/T tree-mean into P so the mm2 PSUM result is final
    P = np.asarray(final_probabilities, dtype=np.float32)[:, LEAF_PERM, :] * (1.0 / T)

    # x [B, F] -> xT [KF, 128, B] (transposed, contraction-chunk major)
    xT = np.ascontiguousarray(x.T).astype(bf).reshape(KF, 128, B)
    # W [T, F, N] -> [F, T, N] -> [KF, 128, TP, 510] -> j-major [TP, 128, KF*510]
    Wr = (
        np.ascontiguousarray(W.transpose(1, 0, 2))
        .astype(bf)
        .reshape(KF, 128, TP, 2 * NODES)
        .transpose(2, 1, 0, 3)
        .reshape(TP, 128, KF * 2 * NODES)
    )
    Wr = np.ascontiguousarray(Wr)
    # P [T, 256, C] -> [leaf-chunk, 128, T*C]
    Pr = np.ascontiguousarray(
        P.reshape(T, 2, 128, C).transpose(1, 2, 0, 3)
    ).astype(bf).reshape(2, 128, T * C)
    return xT, Wr, Pr


def _run(inputs, trace=False, trace_cores=None):
    xT, Wr, Pr = _prep_inputs(
        inputs["l_input"], inputs["cnn_w"], inputs["final_probabilities"]
    )
    in_maps = [
        {
            "xT": np.ascontiguousarray(xT[:, :, c * B_LOC : (c + 1) * B_LOC]),
            "w": Wr,
            "p": Pr,
        }
        for c in range(N_CORES)
    ]
    last_err = None
    for attempt in range(3):
        try:
            res = run_bass_kernel_spmd(
                _get_nc(),
                in_maps,
                core_ids=list(range(N_CORES)),
                trace=trace,
                trace_cores=trace_cores,
            )
            break
        except Exception as e:  # transient NRT device errors: retry
            last_err = e
            if attempt == 2:
                raise
            import time as _time

            _time.sleep(5)
    out = np.concatenate([res.results[c]["out"] for c in range(N_CORES)], axis=0)
    return out, res


def kernel(**inputs) -> np.ndarray:
    out, _ = _run(inputs)
    return out
